# revision 11
# baseline (speedup 1.0000x reference)
"""Trainium2 Bass kernel for a dense transformer block (attention + LoRA +
MLP + proj), data-parallel over batch across 8 NeuronCores.

Contract: kernel(**inputs) takes the FULL unsharded inputs (numpy arrays,
keys as in reference.setup_inputs()) and returns the FULL [8, 512, 1024]
fp32 output.

Design (per core, one batch element):
  - Everything flows channel-major ("transposed"): activations are [C, S]
    tiles with channels on SBUF partitions.  All weights are then used in
    their natural [C_in, C_out] layout (as stationary lhsT slices for
    channel-major outputs, as moving rhs for the token-major v); the only
    transposes in the whole pipeline happen on the host (x -> x.T in,
    out.T -> out).
  - Attention runs keys-on-partitions (attnT = K q^T per head).  The key
    mask is folded into v: masked key ROWS of the token-major v (and of
    its appended ones-columns) are zeroed, which is mathematically
    identical to masking the softmax numerator and denominator.  The
    softmax exp is then a single bias-free ACT op per head, with the
    1/sqrt(hd) scale folded into its scale argument, and the denominator
    comes free as a ones-column appended to v in the PV matmul (M=65).
    Normalization happens per head-pair (overlapped with later heads) via
    a tiny K=2 broadcast matmul of the f32r reciprocals.
  - GEMMs run in bf16 (measured ~2x faster than fp32r per matmul); PSUM
    accumulation is fp32.
"""

import numpy as np

B, S, C = 8, 512, 1024
H, HD, R, HID = 16, 64, 32, 1024
NC3 = 3 * C
NCORES = 8
KC = C // 128          # 8 contraction chunks
MQK = 2 * C // 128     # 16 q+k channel-major output chunks
VSTRIDE = HD + 1       # v columns per head incl. ones column

_cache = {}


def _get_nc():
    if "nc" in _cache:
        return _cache["nc"]

    from contextlib import ExitStack
    import concourse.tile as tile
    from concourse import bacc, mybir

    f32 = mybir.dt.float32
    f32r = mybir.dt.float32r
    bf16 = mybir.dt.bfloat16
    AF = mybir.ActivationFunctionType
    ALU = mybir.AluOpType

    nc = bacc.Bacc("TRN2", target_bir_lowering=False, debug=False)

    def din(name, shape, dt=bf16):
        return nc.dram_tensor(name, list(shape), dt, kind="ExternalInput")

    xT_d = din("xT", (C, S))
    mask01_d = din("mask01", (128, 4), f32)
    sel2_d = din("sel2", (2, 128), f32r)
    qkv_w_d = din("qkv_w", (C, NC3))
    qkv_la_d = din("qkv_la", (C, R))
    qkv_lb_d = din("qkv_lb", (R, NC3))
    proj_w_d = din("proj_w", (C, C))
    proj_b_d = din("proj_b", (C,), f32)
    proj_la_d = din("proj_la", (C, R))
    proj_lb_d = din("proj_lb", (R, C))
    fc1_w_d = din("fc1_w", (C, HID))
    fc1_b_d = din("fc1_b", (HID,), f32)
    fc1_la_d = din("fc1_la", (C, R))
    fc1_lb_d = din("fc1_lb", (R, HID))
    fc2_w_d = din("fc2_w", (HID, C))
    fc2_b_d = din("fc2_b", (C,), f32)
    fc2_la_d = din("fc2_la", (HID, R))
    fc2_lb_d = din("fc2_lb", (R, C))
    outT_d = nc.dram_tensor("outT", [C, S], f32, kind="ExternalOutput")

    with tile.TileContext(nc) as tc, ExitStack() as ctx:
        resident = ctx.enter_context(tc.tile_pool(name="resident", bufs=1))
        wpool = ctx.enter_context(tc.tile_pool(name="wstream", bufs=6))
        psum = ctx.enter_context(tc.tile_pool(name="psum", bufs=2, space="PSUM"))
        expp = ctx.enter_context(tc.tile_pool(name="expp", bufs=2))
        tmpp = ctx.enter_context(tc.tile_pool(name="tmpp", bufs=2))
        outp = ctx.enter_context(tc.tile_pool(name="outp", bufs=2))

        def big_psum(name):
            # [128, 4, S] fp32 = 4 PSUM banks; the only psum tag (2 bufs = all
            # 8 banks).
            return psum.tile([128, 4, S], f32, name=name, tag="big")

        # ---- resident loads -------------------------------------------------
        xT = resident.tile([128, KC, S], bf16, name="xT", tag="xT")
        nc.sync.dma_start(xT[:], xT_d[:].rearrange("(c p) s -> p c s", p=128))
        mask01 = resident.tile([128, 4], f32, name="mask01", tag="mask01")
        nc.sync.dma_start(mask01[:], mask01_d[:])
        sel2 = resident.tile([2, 128], f32r, name="sel2", tag="sel2")
        nc.sync.dma_start(sel2[:], sel2_d[:])

        la = {}
        lb = {}
        for nm, la_d, lb_d, ncols in (
            ("qkv", qkv_la_d, qkv_lb_d, NC3),
            ("fc1", fc1_la_d, fc1_lb_d, HID),
            ("fc2", fc2_la_d, fc2_lb_d, C),
            ("proj", proj_la_d, proj_lb_d, C),
        ):
            la[nm] = resident.tile(
                [128, KC, R], bf16, name=f"la_{nm}", tag=f"la_{nm}"
            )
            nc.sync.dma_start(
                la[nm][:], la_d[:].rearrange("(c p) r -> p c r", p=128)
            )
            lb[nm] = resident.tile(
                [R, ncols], bf16, name=f"lb_{nm}", tag=f"lb_{nm}"
            )
            nc.sync.dma_start(lb[nm][:], lb_d[:])

        biases = {}
        for nm, b_d in (("fc1", fc1_b_d), ("fc2", fc2_b_d), ("proj", proj_b_d)):
            biases[nm] = resident.tile(
                [128, KC], f32, name=f"b_{nm}", tag=f"b_{nm}"
            )
            nc.sync.dma_start(
                biases[nm][:], b_d[:].rearrange("(m p) -> p m", p=128)
            )

        qkv_w_r = qkv_w_d[:].rearrange("(k p) n -> k p n", p=128)
        fc1_w_r = fc1_w_d[:].rearrange("(k p) n -> k p n", p=128)
        fc2_w_r = fc2_w_d[:].rearrange("(k p) n -> k p n", p=128)
        proj_w_r = proj_w_d[:].rearrange("(k p) n -> k p n", p=128)

        def lora_tT(nm, act):
            """tT = (act_rowmajor @ la)^T as a [R, S] tile; act is [128, KC, S]."""
            pt = big_psum(f"pt_{nm}")
            for kc in range(KC):
                nc.tensor.matmul(
                    pt[0:R, 0, :], la[nm][:, kc, :], act[:, kc, :],
                    start=(kc == 0), stop=(kc == KC - 1),
                )
            t = resident.tile([R, S], bf16, name=f"tT_{nm}", tag=f"tT_{nm}")
            nc.any.tensor_copy(t[:], pt[0:R, 0, :])
            return t

        # ---- qkv GEMM -------------------------------------------------------
        tT_qkv = lora_tT("qkv", xT)

        # q,k channel-major: qkT[:, m, :], m in [0,16) covers channels [0,2C)
        qkT = resident.tile([128, MQK, S], bf16, name="qkT", tag="qkT")
        for g in range(4):            # groups of 4 output chunks
            pg = big_psum(f"pqk{g}")
            for kc in range(KC):
                wt = wpool.tile([128, 512], bf16, tag="w")
                nc.sync.dma_start(
                    wt[:], qkv_w_r[kc, :, g * 512:(g + 1) * 512]
                )
                for i in range(4):
                    nc.tensor.matmul(
                        pg[:, i, :], wt[:, i * 128:(i + 1) * 128],
                        xT[:, kc, :], start=(kc == 0), stop=False,
                    )
            for i in range(4):
                m = g * 4 + i
                nc.tensor.matmul(
                    pg[:, i, :], lb["qkv"][:, m * 128:(m + 1) * 128],
                    tT_qkv[:], start=False, stop=True,
                )
            nc.any.tensor_copy(qkT[:, g * 4:(g + 1) * 4, :], pg[:])

        # v token-major with interleaved ones columns: v[:, c, h*65:+64];
        # masked key rows (incl. their ones entries) are zeroed -> the mask
        # needs no separate handling anywhere else.
        v = resident.tile([128, 4, H * VSTRIDE], bf16, name="vtok", tag="vtok")
        for h in range(H):
            nc.vector.memset(
                v[:, :, h * VSTRIDE + HD:h * VSTRIDE + HD + 1], 1.0
            )
        for c in range(4):
            ones_cols = v[:, c, :].rearrange("p (h z) -> p h z", z=VSTRIDE)[
                :, :, HD:HD + 1
            ]
            nc.vector.tensor_scalar_mul(ones_cols, ones_cols, mask01[:, c:c + 1])
        for n in range(2):
            pg = big_psum(f"pv{n}")
            for kc in range(KC):
                wt = wpool.tile([128, 512], bf16, tag="w")
                nc.sync.dma_start(
                    wt[:], qkv_w_r[kc, :, 2 * C + n * 512:2 * C + (n + 1) * 512]
                )
                for c in range(4):
                    nc.tensor.matmul(
                        pg[:, c, :], xT[:, kc, c * 128:(c + 1) * 128],
                        wt[:], start=(kc == 0), stop=False,
                    )
            for c in range(4):
                nc.tensor.matmul(
                    pg[:, c, :], tT_qkv[:, c * 128:(c + 1) * 128],
                    lb["qkv"][:, 2 * C + n * 512:2 * C + (n + 1) * 512],
                    start=False, stop=True,
                )
                # copy 8 heads' columns into 65-strided slots, zeroing masked
                # key rows on the way
                dst = v[:, c, n * 8 * VSTRIDE:(n + 1) * 8 * VSTRIDE].rearrange(
                    "p (h z) -> p h z", z=VSTRIDE
                )[:, :, 0:HD]
                src = pg[:, c, :].rearrange("p (h z) -> p h z", z=HD)
                nc.vector.tensor_scalar_mul(dst, src, mask01[:, c:c + 1])

        # ---- attention ------------------------------------------------------
        # xou: unnormalized attention output, channel-major [128, KC, S]
        xou = resident.tile([128, KC, S], bf16, name="xou", tag="xou")
        den = resident.tile([2, KC, S], f32r, name="den", tag="den")
        recip = resident.tile([2, KC, S], f32r, name="recip", tag="recip")
        for h in range(H):
            j, half = h // 2, h % 2
            p0 = 64 * half
            pa = big_psum("pa")
            for c in range(4):
                nc.tensor.matmul(
                    pa[:, c, :],
                    qkT[p0:p0 + 64, 8 + j, c * 128:(c + 1) * 128],
                    qkT[p0:p0 + 64, j, :],
                )
            exp_t = expp.tile([128, 4, S], bf16, name="exp_t", tag="exp")
            # exp(attn / sqrt(hd)); masking already folded into v
            nc.scalar.activation(exp_t[:], pa[:], AF.Exp, scale=0.125)
            # PV accumulates into pa's bank 0 (free after the exp read);
            # row 64 is the softmax denominator via v's ones column.
            for c in range(4):
                nc.tensor.matmul(
                    pa[0:VSTRIDE, 0, :],
                    v[:, c, h * VSTRIDE:(h + 1) * VSTRIDE],
                    exp_t[:, c, :],
                    start=(c == 0), stop=(c == 3),
                )
            tmd = tmpp.tile([128, S], f32r, name="tmd", tag="tmpd")
            nc.vector.tensor_copy(tmd[HD:HD + 1, :], pa[HD:HD + 1, 0, :])
            nc.sync.dma_start(den[half:half + 1, j, :], tmd[HD:HD + 1, :])
            if half == 0:
                nc.vector.tensor_copy(xou[0:64, j, :], pa[0:HD, 0, :])
            else:
                tmb = tmpp.tile([128, S], bf16, name="tmb", tag="tmpb")
                nc.vector.tensor_copy(tmb[0:HD, :], pa[0:HD, 0, :])
                nc.sync.dma_start(xou[64:128, j, :], tmb[0:HD, :])
                # pair complete: normalize chunk j (overlaps later heads).
                # Broadcast the two per-head reciprocal rows to 64 partitions
                # each via a K=2 matmul into pa's bank 1, then scale xou.
                with nc.allow_low_precision(reason="f32r keeps fp32 bits"):
                    nc.vector.reciprocal(recip[:, j, :], den[:, j, :])
                nc.tensor.matmul(pa[:, 1, :], sel2[:], recip[:, j, :])
                nc.vector.tensor_mul(xou[:, j, :], xou[:, j, :], pa[:, 1, :])
        xoT = xou  # normalized in place

        # ---- MLP fc1 + gelu -------------------------------------------------
        tT_fc1 = lora_tT("fc1", xoT)
        gT = resident.tile([128, KC, S], bf16, name="gT", tag="gT")
        for g in range(2):
            pg = big_psum(f"pf{g}")
            for kc in range(KC):
                wt = wpool.tile([128, 512], bf16, tag="w")
                nc.sync.dma_start(wt[:], fc1_w_r[kc, :, g * 512:(g + 1) * 512])
                for i in range(4):
                    nc.tensor.matmul(
                        pg[:, i, :], wt[:, i * 128:(i + 1) * 128],
                        xoT[:, kc, :], start=(kc == 0), stop=False,
                    )
            for i in range(4):
                m = g * 4 + i
                nc.tensor.matmul(
                    pg[:, i, :], lb["fc1"][:, m * 128:(m + 1) * 128],
                    tT_fc1[:], start=False, stop=True,
                )
                nc.scalar.activation(
                    gT[:, m, :], pg[:, i, :], AF.Gelu,
                    bias=biases["fc1"][:, m:m + 1],
                )

        # ---- MLP fc2 + residual --------------------------------------------
        tT_fc2 = lora_tT("fc2", gT)
        xo2T = resident.tile([128, KC, S], bf16, name="xo2T", tag="xo2T")
        for g in range(2):
            pg = big_psum(f"pg{g}")
            for kc in range(KC):
                wt = wpool.tile([128, 512], bf16, tag="w")
                nc.sync.dma_start(wt[:], fc2_w_r[kc, :, g * 512:(g + 1) * 512])
                for i in range(4):
                    nc.tensor.matmul(
                        pg[:, i, :], wt[:, i * 128:(i + 1) * 128],
                        gT[:, kc, :], start=(kc == 0), stop=False,
                    )
            for i in range(4):
                m = g * 4 + i
                nc.tensor.matmul(
                    pg[:, i, :], lb["fc2"][:, m * 128:(m + 1) * 128],
                    tT_fc2[:], start=False, stop=True,
                )
                # xo2 = (fc2_psum + bias) + xo  (residual)
                nc.vector.scalar_tensor_tensor(
                    xo2T[:, m, :], pg[:, i, :], biases["fc2"][:, m:m + 1],
                    xoT[:, m, :], op0=ALU.add, op1=ALU.add,
                )

        # ---- proj -----------------------------------------------------------
        tT_proj = lora_tT("proj", xo2T)
        outT_r = outT_d[:].rearrange("(m p) s -> p m s", p=128)
        for g in range(2):
            pg = big_psum(f"pp{g}")
            for kc in range(KC):
                wt = wpool.tile([128, 512], bf16, tag="w")
                nc.sync.dma_start(wt[:], proj_w_r[kc, :, g * 512:(g + 1) * 512])
                for i in range(4):
                    nc.tensor.matmul(
                        pg[:, i, :], wt[:, i * 128:(i + 1) * 128],
                        xo2T[:, kc, :], start=(kc == 0), stop=False,
                    )
            ot = outp.tile([128, 4, S], f32, name="ot", tag="out")
            for i in range(4):
                m = g * 4 + i
                nc.tensor.matmul(
                    pg[:, i, :], lb["proj"][:, m * 128:(m + 1) * 128],
                    tT_proj[:], start=False, stop=True,
                )
                nc.scalar.activation(
                    ot[:, i, :], pg[:, i, :], AF.Identity,
                    bias=biases["proj"][:, m:m + 1],
                )
            nc.sync.dma_start(outT_r[:, g * 4:(g + 1) * 4, :], ot[:])

    nc.compile()
    _cache["nc"] = nc
    return nc


def _bf16(a):
    import ml_dtypes

    return np.asarray(a, dtype=np.float32).astype(ml_dtypes.bfloat16)


def _make_in_maps(inputs):
    x = np.asarray(inputs["x"], dtype=np.float32)
    mask = np.asarray(inputs["mask"])
    sel2 = np.zeros((2, 128), dtype=np.float32)
    sel2[0, 0:64] = 1.0
    sel2[1, 64:128] = 1.0
    shared = {"sel2": sel2}
    for k in (
        "qkv_w", "qkv_la", "qkv_lb", "proj_w", "proj_la", "proj_lb",
        "fc1_w", "fc1_la", "fc1_lb", "fc2_w", "fc2_la", "fc2_lb",
    ):
        shared[k] = np.ascontiguousarray(_bf16(inputs[k]))
    for k in ("proj_b", "fc1_b", "fc2_b"):
        shared[k] = np.ascontiguousarray(inputs[k], dtype=np.float32)
    in_maps = []
    for b in range(NCORES):
        m01 = mask[b, :S].astype(np.float32)          # 1.0 keep / 0.0 drop
        in_maps.append(
            dict(
                shared,
                xT=np.ascontiguousarray(_bf16(x[b].T)),
                mask01=np.ascontiguousarray(m01.reshape(4, 128).T),
            )
        )
    return in_maps


def _run(inputs, trace=False):
    from concourse.bass_utils import run_bass_kernel_spmd

    nc = _get_nc()
    in_maps = _make_in_maps(inputs)
    res = run_bass_kernel_spmd(nc, in_maps, list(range(NCORES)), trace=trace)
    out = np.stack(
        [np.ascontiguousarray(res.results[b]["outT"].T) for b in range(NCORES)]
    )
    return out, res


def kernel(**inputs):
    out, _ = _run(inputs, trace=False)
    return out


# revision 12
# speedup vs baseline: 1.0223x; 1.0223x over previous
"""Trainium2 Bass kernel for a dense transformer block (attention + LoRA +
MLP + proj), data-parallel over batch across 8 NeuronCores.

Contract: kernel(**inputs) takes the FULL unsharded inputs (numpy arrays,
keys as in reference.setup_inputs()) and returns the FULL [8, 512, 1024]
fp32 output.

Design (per core, one batch element):
  - Everything flows channel-major ("transposed"): activations are [C, S]
    tiles with channels on SBUF partitions.  All weights are then used in
    their natural [C_in, C_out] layout (as stationary lhsT slices for
    channel-major outputs, as moving rhs for the token-major v); the only
    transposes in the whole pipeline happen on the host (x -> x.T in,
    out.T -> out).
  - Attention runs keys-on-partitions (attnT = K q^T per head).  The key
    mask is folded into v: masked key ROWS of the token-major v (and of
    its appended ones-columns) are zeroed, which is mathematically
    identical to masking the softmax numerator and denominator.  The
    softmax exp is then a single bias-free ACT op per head, with the
    1/sqrt(hd) scale folded into its scale argument, and the denominator
    comes free as a ones-column appended to v in the PV matmul (M=65).
    Normalization happens per head-pair (overlapped with later heads) via
    a tiny K=2 broadcast matmul of the f32r reciprocals.
  - GEMMs run in bf16 (measured ~2x faster than fp32r per matmul); PSUM
    accumulation is fp32.
"""

import numpy as np

B, S, C = 8, 512, 1024
H, HD, R, HID = 16, 64, 32, 1024
NC3 = 3 * C
NCORES = 8
KC = C // 128          # 8 contraction chunks
MQK = 2 * C // 128     # 16 q+k channel-major output chunks
VSTRIDE = HD + 1       # v columns per head incl. ones column

_cache = {}


def _get_nc():
    if "nc" in _cache:
        return _cache["nc"]

    from contextlib import ExitStack
    import concourse.tile as tile
    from concourse import bacc, mybir

    f32 = mybir.dt.float32
    f32r = mybir.dt.float32r
    bf16 = mybir.dt.bfloat16
    AF = mybir.ActivationFunctionType
    ALU = mybir.AluOpType

    nc = bacc.Bacc("TRN2", target_bir_lowering=False, debug=False)

    def din(name, shape, dt=bf16):
        return nc.dram_tensor(name, list(shape), dt, kind="ExternalInput")

    xT_d = din("xT", (C, S))
    mask01_d = din("mask01", (128, 4), f32)
    sel2_d = din("sel2", (2, 128), f32r)
    qkv_w_d = din("qkv_w", (C, NC3))
    qkv_la_d = din("qkv_la", (C, R))
    qkv_lb_d = din("qkv_lb", (R, NC3))
    proj_w_d = din("proj_w", (C, C))
    proj_b_d = din("proj_b", (C,), f32)
    proj_la_d = din("proj_la", (C, R))
    proj_lb_d = din("proj_lb", (R, C))
    fc1_w_d = din("fc1_w", (C, HID))
    fc1_b_d = din("fc1_b", (HID,), f32)
    fc1_la_d = din("fc1_la", (C, R))
    fc1_lb_d = din("fc1_lb", (R, HID))
    fc2_w_d = din("fc2_w", (HID, C))
    fc2_b_d = din("fc2_b", (C,), f32)
    fc2_la_d = din("fc2_la", (HID, R))
    fc2_lb_d = din("fc2_lb", (R, C))
    outT_d = nc.dram_tensor("outT", [C, S], f32, kind="ExternalOutput")

    with tile.TileContext(nc) as tc, ExitStack() as ctx:
        resident = ctx.enter_context(tc.tile_pool(name="resident", bufs=1))
        wpool = ctx.enter_context(tc.tile_pool(name="wstream", bufs=6))
        psum = ctx.enter_context(tc.tile_pool(name="psum", bufs=2, space="PSUM"))
        expp = ctx.enter_context(tc.tile_pool(name="expp", bufs=2))
        tmpp = ctx.enter_context(tc.tile_pool(name="tmpp", bufs=2))
        outp = ctx.enter_context(tc.tile_pool(name="outp", bufs=2))

        def big_psum(name):
            # [128, 4, S] fp32 = 4 PSUM banks; the only psum tag (2 bufs = all
            # 8 banks).
            return psum.tile([128, 4, S], f32, name=name, tag="big")

        # ---- resident loads -------------------------------------------------
        xT = resident.tile([128, KC, S], bf16, name="xT", tag="xT")
        nc.sync.dma_start(xT[:], xT_d[:].rearrange("(c p) s -> p c s", p=128))
        mask01 = resident.tile([128, 4], f32, name="mask01", tag="mask01")
        nc.sync.dma_start(mask01[:], mask01_d[:])
        sel2 = resident.tile([2, 128], f32r, name="sel2", tag="sel2")
        nc.sync.dma_start(sel2[:], sel2_d[:])

        la = {}
        lb = {}
        for nm, la_d, lb_d, ncols in (
            ("qkv", qkv_la_d, qkv_lb_d, NC3),
            ("fc1", fc1_la_d, fc1_lb_d, HID),
            ("fc2", fc2_la_d, fc2_lb_d, C),
            ("proj", proj_la_d, proj_lb_d, C),
        ):
            la[nm] = resident.tile(
                [128, KC, R], bf16, name=f"la_{nm}", tag=f"la_{nm}"
            )
            nc.sync.dma_start(
                la[nm][:], la_d[:].rearrange("(c p) r -> p c r", p=128)
            )
            lb[nm] = resident.tile(
                [R, ncols], bf16, name=f"lb_{nm}", tag=f"lb_{nm}"
            )
            nc.sync.dma_start(lb[nm][:], lb_d[:])

        biases = {}
        for nm, b_d in (("fc1", fc1_b_d), ("fc2", fc2_b_d), ("proj", proj_b_d)):
            biases[nm] = resident.tile(
                [128, KC], f32, name=f"b_{nm}", tag=f"b_{nm}"
            )
            nc.sync.dma_start(
                biases[nm][:], b_d[:].rearrange("(m p) -> p m", p=128)
            )

        qkv_w_r = qkv_w_d[:].rearrange("(k p) n -> k p n", p=128)
        fc1_w_r = fc1_w_d[:].rearrange("(k p) n -> k p n", p=128)
        fc2_w_r = fc2_w_d[:].rearrange("(k p) n -> k p n", p=128)
        proj_w_r = proj_w_d[:].rearrange("(k p) n -> k p n", p=128)

        def lora_tT(nm, act):
            """tT = (act_rowmajor @ la)^T as a [R, S] tile; act is [128, KC, S]."""
            pt = big_psum(f"pt_{nm}")
            for kc in range(KC):
                nc.tensor.matmul(
                    pt[0:R, 0, :], la[nm][:, kc, :], act[:, kc, :],
                    start=(kc == 0), stop=(kc == KC - 1),
                )
            t = resident.tile([R, S], bf16, name=f"tT_{nm}", tag=f"tT_{nm}")
            nc.any.tensor_copy(t[:], pt[0:R, 0, :])
            return t

        # ---- qkv GEMM -------------------------------------------------------
        tT_qkv = lora_tT("qkv", xT)

        # q,k channel-major: qkT[:, m, :], m in [0,16) covers channels [0,2C)
        qkT = resident.tile([128, MQK, S], bf16, name="qkT", tag="qkT")
        for g in range(4):            # groups of 4 output chunks
            pg = big_psum(f"pqk{g}")
            for kc in range(KC):
                wt = wpool.tile([128, 512], bf16, tag="w")
                nc.sync.dma_start(
                    wt[:], qkv_w_r[kc, :, g * 512:(g + 1) * 512]
                )
                for i in range(4):
                    nc.tensor.matmul(
                        pg[:, i, :], wt[:, i * 128:(i + 1) * 128],
                        xT[:, kc, :], start=(kc == 0), stop=False,
                    )
            for i in range(4):
                m = g * 4 + i
                nc.tensor.matmul(
                    pg[:, i, :], lb["qkv"][:, m * 128:(m + 1) * 128],
                    tT_qkv[:], start=False, stop=True,
                )
            nc.any.tensor_copy(qkT[:, g * 4:(g + 1) * 4, :], pg[:])

        # v token-major with interleaved ones columns: v[:, c, h*65:+64];
        # masked key rows (incl. their ones entries) are zeroed -> the mask
        # needs no separate handling anywhere else.
        v = resident.tile([128, 4, H * VSTRIDE], bf16, name="vtok", tag="vtok")
        for h in range(H):
            nc.vector.memset(
                v[:, :, h * VSTRIDE + HD:h * VSTRIDE + HD + 1], 1.0
            )
        for c in range(4):
            ones_cols = v[:, c, :].rearrange("p (h z) -> p h z", z=VSTRIDE)[
                :, :, HD:HD + 1
            ]
            nc.vector.tensor_scalar_mul(ones_cols, ones_cols, mask01[:, c:c + 1])
        for n in range(2):
            pg = big_psum(f"pv{n}")
            for kc in range(KC):
                wt = wpool.tile([128, 512], bf16, tag="w")
                nc.sync.dma_start(
                    wt[:], qkv_w_r[kc, :, 2 * C + n * 512:2 * C + (n + 1) * 512]
                )
                for c in range(4):
                    nc.tensor.matmul(
                        pg[:, c, :], xT[:, kc, c * 128:(c + 1) * 128],
                        wt[:], start=(kc == 0), stop=False,
                    )
            for c in range(4):
                nc.tensor.matmul(
                    pg[:, c, :], tT_qkv[:, c * 128:(c + 1) * 128],
                    lb["qkv"][:, 2 * C + n * 512:2 * C + (n + 1) * 512],
                    start=False, stop=True,
                )
                # copy 8 heads' columns into 65-strided slots, zeroing masked
                # key rows on the way
                dst = v[:, c, n * 8 * VSTRIDE:(n + 1) * 8 * VSTRIDE].rearrange(
                    "p (h z) -> p h z", z=VSTRIDE
                )[:, :, 0:HD]
                src = pg[:, c, :].rearrange("p (h z) -> p h z", z=HD)
                nc.vector.tensor_scalar_mul(dst, src, mask01[:, c:c + 1])

        # ---- attention ------------------------------------------------------
        # xou: unnormalized attention output, channel-major [128, KC, S]
        xou = resident.tile([128, KC, S], bf16, name="xou", tag="xou")
        den = resident.tile([2, KC, S], f32r, name="den", tag="den")
        recip = resident.tile([2, KC, S], f32r, name="recip", tag="recip")
        for h in range(H):
            j, half = h // 2, h % 2
            p0 = 64 * half
            pa = big_psum("pa")
            for c in range(4):
                nc.tensor.matmul(
                    pa[:, c, :],
                    qkT[p0:p0 + 64, 8 + j, c * 128:(c + 1) * 128],
                    qkT[p0:p0 + 64, j, :],
                )
            exp_t = expp.tile([128, 4, S], bf16, name="exp_t", tag="exp")
            # exp(attn / sqrt(hd)); masking already folded into v
            nc.scalar.activation(exp_t[:], pa[:], AF.Exp, scale=0.125)
            # PV accumulates into pa's bank 0 (free after the exp read);
            # row 64 is the softmax denominator via v's ones column.
            for c in range(4):
                nc.tensor.matmul(
                    pa[0:VSTRIDE, 0, :],
                    v[:, c, h * VSTRIDE:(h + 1) * VSTRIDE],
                    exp_t[:, c, :],
                    start=(c == 0), stop=(c == 3),
                )
            tmd = tmpp.tile([128, S], f32r, name="tmd", tag="tmpd")
            nc.vector.tensor_copy(tmd[HD:HD + 1, :], pa[HD:HD + 1, 0, :])
            nc.sync.dma_start(den[half:half + 1, j, :], tmd[HD:HD + 1, :])
            if half == 0:
                nc.vector.tensor_copy(xou[0:64, j, :], pa[0:HD, 0, :])
            else:
                tmb = tmpp.tile([128, S], bf16, name="tmb", tag="tmpb")
                nc.vector.tensor_copy(tmb[0:HD, :], pa[0:HD, 0, :])
                nc.sync.dma_start(xou[64:128, j, :], tmb[0:HD, :])
        # normalize: one reciprocal for all heads (DVE reciprocal has a large
        # fixed cost), then per chunk a K=2 broadcast matmul + scale; chunk j
        # unblocks fc1's kc=j work via subtile deps.
        with nc.allow_low_precision(reason="f32r keeps fp32 bits"):
            nc.vector.reciprocal(recip[:], den[:])
        for j in range(KC):
            pn = big_psum(f"pn{j}")
            nc.tensor.matmul(pn[:, 0, :], sel2[:], recip[:, j, :])
            nc.vector.tensor_mul(xou[:, j, :], xou[:, j, :], pn[:, 0, :])
        xoT = xou  # normalized in place

        # ---- MLP fc1 + gelu -------------------------------------------------
        tT_fc1 = lora_tT("fc1", xoT)
        gT = resident.tile([128, KC, S], bf16, name="gT", tag="gT")
        for g in range(2):
            pg = big_psum(f"pf{g}")
            for kc in range(KC):
                wt = wpool.tile([128, 512], bf16, tag="w")
                nc.sync.dma_start(wt[:], fc1_w_r[kc, :, g * 512:(g + 1) * 512])
                for i in range(4):
                    nc.tensor.matmul(
                        pg[:, i, :], wt[:, i * 128:(i + 1) * 128],
                        xoT[:, kc, :], start=(kc == 0), stop=False,
                    )
            for i in range(4):
                m = g * 4 + i
                nc.tensor.matmul(
                    pg[:, i, :], lb["fc1"][:, m * 128:(m + 1) * 128],
                    tT_fc1[:], start=False, stop=True,
                )
                nc.scalar.activation(
                    gT[:, m, :], pg[:, i, :], AF.Gelu,
                    bias=biases["fc1"][:, m:m + 1],
                )

        # ---- MLP fc2 + residual --------------------------------------------
        tT_fc2 = lora_tT("fc2", gT)
        xo2T = resident.tile([128, KC, S], bf16, name="xo2T", tag="xo2T")
        for g in range(2):
            pg = big_psum(f"pg{g}")
            for kc in range(KC):
                wt = wpool.tile([128, 512], bf16, tag="w")
                nc.sync.dma_start(wt[:], fc2_w_r[kc, :, g * 512:(g + 1) * 512])
                for i in range(4):
                    nc.tensor.matmul(
                        pg[:, i, :], wt[:, i * 128:(i + 1) * 128],
                        gT[:, kc, :], start=(kc == 0), stop=False,
                    )
            for i in range(4):
                m = g * 4 + i
                nc.tensor.matmul(
                    pg[:, i, :], lb["fc2"][:, m * 128:(m + 1) * 128],
                    tT_fc2[:], start=False, stop=True,
                )
                # xo2 = (fc2_psum + bias) + xo  (residual)
                nc.vector.scalar_tensor_tensor(
                    xo2T[:, m, :], pg[:, i, :], biases["fc2"][:, m:m + 1],
                    xoT[:, m, :], op0=ALU.add, op1=ALU.add,
                )

        # ---- proj -----------------------------------------------------------
        tT_proj = lora_tT("proj", xo2T)
        outT_r = outT_d[:].rearrange("(m p) s -> p m s", p=128)
        for g in range(2):
            pg = big_psum(f"pp{g}")
            for kc in range(KC):
                wt = wpool.tile([128, 512], bf16, tag="w")
                nc.sync.dma_start(wt[:], proj_w_r[kc, :, g * 512:(g + 1) * 512])
                for i in range(4):
                    nc.tensor.matmul(
                        pg[:, i, :], wt[:, i * 128:(i + 1) * 128],
                        xo2T[:, kc, :], start=(kc == 0), stop=False,
                    )
            ot = outp.tile([128, 4, S], f32, name="ot", tag="out")
            for i in range(4):
                m = g * 4 + i
                nc.tensor.matmul(
                    pg[:, i, :], lb["proj"][:, m * 128:(m + 1) * 128],
                    tT_proj[:], start=False, stop=True,
                )
                nc.scalar.activation(
                    ot[:, i, :], pg[:, i, :], AF.Identity,
                    bias=biases["proj"][:, m:m + 1],
                )
            nc.sync.dma_start(outT_r[:, g * 4:(g + 1) * 4, :], ot[:])

    nc.compile()
    _cache["nc"] = nc
    return nc


def _bf16(a):
    import ml_dtypes

    return np.asarray(a, dtype=np.float32).astype(ml_dtypes.bfloat16)


def _make_in_maps(inputs):
    x = np.asarray(inputs["x"], dtype=np.float32)
    mask = np.asarray(inputs["mask"])
    sel2 = np.zeros((2, 128), dtype=np.float32)
    sel2[0, 0:64] = 1.0
    sel2[1, 64:128] = 1.0
    shared = {"sel2": sel2}
    for k in (
        "qkv_w", "qkv_la", "qkv_lb", "proj_w", "proj_la", "proj_lb",
        "fc1_w", "fc1_la", "fc1_lb", "fc2_w", "fc2_la", "fc2_lb",
    ):
        shared[k] = np.ascontiguousarray(_bf16(inputs[k]))
    for k in ("proj_b", "fc1_b", "fc2_b"):
        shared[k] = np.ascontiguousarray(inputs[k], dtype=np.float32)
    in_maps = []
    for b in range(NCORES):
        m01 = mask[b, :S].astype(np.float32)          # 1.0 keep / 0.0 drop
        in_maps.append(
            dict(
                shared,
                xT=np.ascontiguousarray(_bf16(x[b].T)),
                mask01=np.ascontiguousarray(m01.reshape(4, 128).T),
            )
        )
    return in_maps


def _run(inputs, trace=False):
    from concourse.bass_utils import run_bass_kernel_spmd

    nc = _get_nc()
    in_maps = _make_in_maps(inputs)
    res = run_bass_kernel_spmd(nc, in_maps, list(range(NCORES)), trace=trace)
    out = np.stack(
        [np.ascontiguousarray(res.results[b]["outT"].T) for b in range(NCORES)]
    )
    return out, res


def kernel(**inputs):
    out, _ = _run(inputs, trace=False)
    return out
